# revision 74
# baseline (speedup 1.0000x reference)
"""ContactsFittingLoss on 8 Trainium2 NeuronCores (Bass/Tile).

Row-parallel: verts (N=16384) split across 8 cores; obj_pts, anchors and
the 32 gaussians replicated. Spatial pruning: verts are median-split into
128 spatially-compact tiles of 128; each tile's kNN candidates are the
obj points inside the tile bbox expanded by the tile's exact 5th-NN
radius (host-verified guarantee: every vert has >= K obj points within
the radius), so the pruned top-K is exact. Per core:
  - main loop: per-tile [13 x 128] x [13 x C_t] bf16 hi/lo matmuls
    (-d^2 into PSUM) + DVE max8 top-K, variable 256/512-col chunks,
  - anchor weights via one penalized full-N scan (no collective: an
    8-core AllReduce has a ~90 us fixed latency floor here, above this
    kernel's total runtime): block-diagonal f32r matmuls compute
    Hg = -(maha_a + PEN*(score_a - rowmin)) for all 32 gaussians over
    all 16384 verts, so a per-anchor max of Hg gives the global group
    max of w = exp(-0.5*maha) and, on the core's own block (rotated to
    scan position 0), a row max gives each vert's selected -maha; the
    norm lookup rn[argmin] uses the same penalty trick at PEN2 scale,
  - normalize/threshold, per-partition partials.
Host packs operands and sums the 8x128 partials into the mean.
"""
import numpy as np
import ml_dtypes
import orjson

import concourse.bass as bass
import concourse.mybir as mybir
from concourse.tile import TileContext
from concourse.bass_utils import run_bass_kernel_spmd

F32 = mybir.dt.float32
FR = mybir.dt.float32r
BF16 = mybir.dt.bfloat16
NA = 32
LOG_2PI = float(np.log(2.0 * np.pi))
NCORES = 8
SENTINEL = 10.0
PEN = float(2 ** 20)
PEN2 = float(2 ** 14)

# ---------------------------------------------------------------------------
# Workaround: this container's walrus rejects instructions with >1 sync wait;
# Tile occasionally emits more. Split extras onto NoOps at serialization.
# ---------------------------------------------------------------------------
_uid = [0]


def _split_waits(d):
    for f in d.get('functions', []):
        for blk in f.get('blocks', []):
            out = []
            for ins in blk.get('instructions', []):
                si = ins.get('sync_info')
                ow = (si or {}).get('on_wait') or []
                if len(ow) > 1:
                    for w in ow[:-1]:
                        _uid[0] += 1
                        out.append({'debug': ins.get('debug', 0),
                                    'engine': ins['engine'],
                                    'ins': [], 'outs': [],
                                    'name': f"I-waitsplit-{_uid[0]}",
                                    'opcode': 'NoOp',
                                    'sync_info': {'on_update': [],
                                                  'on_wait': [w]}})
                    si['on_wait'] = ow[-1:]
                out.append(ins)
            blk['instructions'] = out
    return d


if not getattr(bass.Bass, '_cf_waitsplit', False):
    _orig_tjb = bass.Bass.to_json_bytes

    def _patched_tjb(self):
        return orjson.dumps(_split_waits(orjson.loads(_orig_tjb(self))))

    bass.Bass.to_json_bytes = _patched_tjb
    bass.Bass._cf_waitsplit = True


# ---------------------------------------------------------------------------
# Host-side operand packing (marshalling + candidate index construction)
# ---------------------------------------------------------------------------
def _to_bf16(x):
    return np.asarray(x, np.float32).astype(ml_dtypes.bfloat16)


def _hi_lo(x):
    h = _to_bf16(x)
    l = _to_bf16(np.asarray(x, np.float32) - h.astype(np.float32))
    return h, l


def _tile_split(V, idx, depth):
    if depth == 0:
        return [idx]
    pts = V[idx]
    ax = int(np.argmax(pts.max(0) - pts.min(0)))
    order = idx[np.argsort(pts[:, ax], kind='stable')]
    h = len(order) // 2
    return _tile_split(V, order[:h], depth - 1) + _tile_split(V, order[h:], depth - 1)


def _encode_cand(pts, width):
    """Encode candidate obj points as the 13-row bf16 -d^2 rhs block."""
    yp = np.full((width, 3), SENTINEL, np.float32)
    yp[:len(pts)] = pts
    y2 = (yp ** 2).sum(-1)
    yh, yl = _hi_lo(yp.T)
    y2h, y2l = _hi_lo(y2)
    blk = np.zeros((13, width), ml_dtypes.bfloat16)
    blk[0:3] = yh
    blk[3:6] = yl
    blk[6:9] = yh
    blk[9] = y2h
    blk[10] = y2l
    blk[11] = 1.0
    blk[12] = 1.0
    return blk


def _host_prep(verts, anchor_verts, obj_pts, contact_gaussians, K):
    V0 = np.asarray(verts[0], np.float32)
    Y = np.asarray(obj_pts[0], np.float32)
    A = np.asarray(anchor_verts[0], np.float32)
    cg = np.asarray(contact_gaussians, np.float32)
    N, P = V0.shape[0], Y.shape[0]
    n_tiles = N // 128
    depth = int(round(np.log2(n_tiles)))
    assert 128 << depth == N
    TT = n_tiles // NCORES

    tiles = _tile_split(V0, np.arange(N), depth)

    # candidate sets: start from bbox + r_pad with the >=K-within-r_pad
    # guarantee, then shrink to the tile's exact max 5th-NN radius.
    cand_of = {}
    for ti in range(n_tiles):
        vt = V0[tiles[ti]]
        r_pad = 0.016
        for _ in range(20):
            lo = vt.min(0) - r_pad
            hi = vt.max(0) + r_pad
            ci = np.nonzero(np.all((Y >= lo) & (Y <= hi), axis=1))[0]
            d2 = ((vt[:, None, :] - Y[ci][None, :, :]) ** 2).sum(-1)
            if len(ci) >= K and (d2 <= r_pad * r_pad).sum(1).min() >= K:
                break
            r_pad *= 1.3
        else:
            raise RuntimeError("candidate radius search failed")
        rt = float(np.sqrt(np.partition(d2, K - 1, axis=1)[:, K - 1]).max())
        rt *= 1.0000002
        lo = vt.min(0) - rt
        hi = vt.max(0) + rt
        ci = np.nonzero(np.all((Y >= lo) & (Y <= hi), axis=1))[0]
        cand_of[ti] = ci

    # order each core's tiles by descending candidate count so the
    # per-position max across cores (the shared SPMD layout) is tight
    order = []
    for c in range(NCORES):
        ids = list(range(c * TT, (c + 1) * TT))
        ids.sort(key=lambda ti: -len(cand_of[ti]))
        order.extend(ids)
    tiles = [tiles[ti] for ti in order]
    cands = [cand_of[ti] for ti in order]
    perm = np.concatenate(tiles)
    V = V0[perm]

    cnt = np.array([(len(c) + 255) // 256 * 256 for c in cands]).reshape(NCORES, TT)
    layout = tuple(int(x) for x in cnt.max(0))
    offs = np.concatenate([[0], np.cumsum(layout)]).astype(int)
    CT = int(offs[-1])

    # candidate rhs blocks [13, NCORES * CT] bf16
    cand = np.zeros((13, NCORES * CT), ml_dtypes.bfloat16)
    for c in range(NCORES):
        for t in range(TT):
            ci = cands[c * TT + t]
            cand[:, c * CT + offs[t]:c * CT + offs[t + 1]] = \
                _encode_cand(Y[ci], layout[t])

    # verts lhs [13, N] bf16
    v2 = (V ** 2).sum(-1)
    vh, vl = _hi_lo(2.0 * V.T)
    v2h, v2l = _hi_lo(v2)
    lhsb = np.zeros((13, N), ml_dtypes.bfloat16)
    lhsb[0:3] = vh
    lhsb[3:6] = vh
    lhsb[6:9] = vl
    lhsb[9] = -1.0
    lhsb[10] = -1.0
    lhsb[11] = -v2h
    lhsb[12] = -v2l

    # gaussian tables
    zero_g = np.all(cg == 0.0, axis=-1)
    means = cg[:, :3] + A
    covs = cg[:, 3:].reshape(NA, 3, 3)
    covs_safe = np.where(zero_g[:, None, None], np.eye(3, dtype=np.float32), covs)
    chol = np.linalg.cholesky(covs_safe)
    logdet = 2.0 * np.sum(np.log(np.diagonal(chol, axis1=-2, axis2=-1)), -1)
    inv = np.linalg.inv(covs_safe)
    theta = np.zeros((NA, 10), np.float32)
    theta[:, 0] = inv[:, 0, 0]
    theta[:, 1] = inv[:, 1, 1]
    theta[:, 2] = inv[:, 2, 2]
    theta[:, 3] = 2.0 * inv[:, 0, 1]
    theta[:, 4] = 2.0 * inv[:, 1, 2]
    theta[:, 5] = 2.0 * inv[:, 0, 2]
    theta[:, 6:9] = -2.0 * np.einsum('kij,kj->ki', inv, means)
    theta[:, 9] = (np.einsum('ki,kij,kj->k', means, inv, means) + logdet
                   + 3.0 * LOG_2PI + np.where(zero_g, 1e4, 0.0))
    anch4 = np.concatenate([-2.0 * A.T, (A * A).sum(-1)[None, :]], 0)  # [4,32]

    # block-diagonal rhs (shared by all cores / groups). mhp fuses the
    # mahalanobis with a scaled anchor-score penalty: H = maha + PEN*sc,
    # so after subtracting PEN*rowmin only the argmin anchor's H stays
    # at maha scale and the per-group min of H yields the group max of
    # w = exp(-0.5*maha).
    screr = np.zeros((4 * TT, TT * NA), np.float32)
    mhp = np.zeros((14 * 8, TT * NA), np.float32)
    for t in range(TT):
        screr[4 * t:4 * t + 4, NA * t:NA * (t + 1)] = anch4
        half, u = divmod(t, 8)
        # mhp columns are anchor-major within each half (col = 8a + u)
        # so the per-group slot reduce reads contiguously; rows negated
        # (the scan computes -maha - PEN*score)
        mhp[14 * u:14 * u + 10, 256 * half + u::8][:, :NA] = -theta.T
        mhp[14 * u + 10:14 * u + 14, 256 * half + u::8][:, :NA] = -PEN * anch4

    # per-core psi/phi stationaries + full-N concatenations
    psis, phipss = [], []
    for c in range(NCORES):
        Vc = V[c * TT * 128:(c + 1) * TT * 128]
        psib = np.zeros((4 * TT, 128), np.float32)
        phips = np.zeros((112, 256), np.float32)
        for t in range(TT):
            vt = Vc[t * 128:(t + 1) * 128]
            psib[4 * t:4 * t + 3] = vt.T
            psib[4 * t + 3] = 1.0
            h, u = divmod(t, 8)
            phi = np.stack([vt[:, 0] ** 2, vt[:, 1] ** 2, vt[:, 2] ** 2,
                            vt[:, 0] * vt[:, 1], vt[:, 1] * vt[:, 2],
                            vt[:, 0] * vt[:, 2],
                            vt[:, 0], vt[:, 1], vt[:, 2],
                            np.ones(128, np.float32)], 0)
            phips[14 * u:14 * u + 10, 128 * h:128 * (h + 1)] = phi
            phips[14 * u + 10:14 * u + 13, 128 * h:128 * (h + 1)] = vt.T
            phips[14 * u + 13, 128 * h:128 * (h + 1)] = 1.0
        psis.append(psib)
        phipss.append(phips)
    # per-core block rotation: block 0 of the full-N scan is always the
    # core's own verts (so its penalized H doubles as the own-anchor
    # select), and the global min over blocks is order-invariant
    psialls, phipsalls = [], []
    for c in range(NCORES):
        rot = [(c + j) % NCORES for j in range(NCORES)]
        psialls.append(np.ascontiguousarray(
            np.concatenate([psis[r] for r in rot], 1)))
        phipsalls.append(np.ascontiguousarray(
            np.concatenate([phipss[r] for r in rot], 1)))
    return dict(V=V, N=N, P=P, layout=layout, CT=CT, cand=cand, lhsb=lhsb,
                screr=screr, mhp=mhp,
                psialls=psialls, phipsalls=phipsalls)


def _pack_core(prep, core, R):
    lo = core * R
    CT = prep["CT"]
    return {
        "candl": np.ascontiguousarray(np.concatenate(
            [prep["lhsb"][:, lo:lo + R],
             prep["cand"][:, core * CT:(core + 1) * CT]], 1)),
        "scpsi": np.ascontiguousarray(np.concatenate(
            [prep["screr"], prep["psialls"][core]], 1)),
        "mhpall": np.ascontiguousarray(np.concatenate(
            [prep["mhp"], prep["phipsalls"][core]], 1)),
    }


# ---------------------------------------------------------------------------
# Device program
# ---------------------------------------------------------------------------
def _build_kernel(R=2048, layout=(), K=5, n_cores=8):
    T = R // 128          # vert tiles per core
    TA = T * NA           # 512
    CT = int(sum(layout))
    offs = [0]
    for w in layout:
        offs.append(offs[-1] + w)
    chunks = []
    for w in layout:
        ch = [512] * (w // 512)
        if w % 512:
            ch.append(w % 512)
        chunks.append(ch)
    nc = bass.Bass(num_devices=n_cores)

    candl_d = nc.dram_tensor("candl", [13, R + CT], BF16,
                             kind="ExternalInput")
    scpsi_d = nc.dram_tensor("scpsi", [4 * T, TA + n_cores * 128], FR,
                             kind="ExternalInput")
    mhpall_d = nc.dram_tensor("mhpall", [112, TA + n_cores * 256], FR,
                              kind="ExternalInput")

    part_d = nc.dram_tensor("part", [128], F32, kind="ExternalOutput")
    warm_d = nc.dram_tensor("warm", [128], F32)

    AX = mybir.AxisListType.X
    OP = mybir.AluOpType
    EXP = mybir.ActivationFunctionType.Exp

    with TileContext(nc) as tc:
        with tc.tile_pool(name="const", bufs=1) as cp:
            candl = cp.tile([13, R + CT], BF16, tag="candl")
            scpsi = cp.tile([4 * T, TA + n_cores * 128], FR, tag="scpsi")
            mhpall = cp.tile([112, TA + n_cores * 256], FR, tag="mhpall")
            ones = cp.tile([1, 128], F32, tag="ones")
            W = cp.tile([128, T], F32, tag="W")
            S5 = cp.tile([128, T], F32, tag="S5")
            dsc = cp.tile([128, TA], F32, tag="dsc")
            smina = cp.tile([128, n_cores * NA], F32, tag="smina")

            nc.vector.memset(ones[:], 1.0)
            c0 = R + offs[2]
            c1 = R + offs[6]
            s1 = TA + 128
            m1 = TA + 256
            m2 = TA + 1024
            nc.sync.dma_start(candl[:, 0:c0], candl_d[:, 0:c0])
            nc.sync.dma_start(scpsi[:, 0:s1], scpsi_d[:, 0:s1])
            nc.sync.dma_start(candl[:, c0:c1], candl_d[:, c0:c1])
            nc.sync.dma_start(mhpall[:, 0:m1], mhpall_d[:, 0:m1])
            nc.sync.dma_start(candl[:, c1:], candl_d[:, c1:])
            nc.sync.dma_start(scpsi[:, s1:], scpsi_d[:, s1:])
            nc.sync.dma_start(mhpall[:, m1:m2], mhpall_d[:, m1:m2])
            nc.sync.dma_start(mhpall[:, m2:], mhpall_d[:, m2:])
            # warm the SBUF->DRAM DMA path: the final 512 B partials write
            # otherwise pays a ~8 us first-use completion latency.
            # Write part_d itself (garbage, overwritten by the real write
            # later on the same queue) in case the latency is per-target.
            nc.sync.dma_start(warm_d[:], ones[0, 0:128])
            nc.sync.dma_start(part_d[:], ones[0, 0:128])

            with tc.tile_pool(name="ps", bufs=2, space="PSUM") as ps, \
                 tc.tile_pool(name="psm", bufs=3, space="PSUM") as psm, \
                 tc.tile_pool(name="psx", bufs=1, space="PSUM") as psx, \
                 tc.tile_pool(name="an", bufs=1) as an, \
                 tc.tile_pool(name="gs", bufs=3) as gs, \
                 tc.tile_pool(name="cnd", bufs=3) as cnd, \
                 tc.tile_pool(name="tl", bufs=1) as tl:

                # -------- interleaved: global group-max scan + main loop
                # (block 0 of the rotated scan = this core's own verts;
                # its penalized H doubles as the own-anchor select)
                def emit_group(g):
                    scg = ps.tile([128, TA], F32, tag="scg")
                    pso = TA + g * 128
                    nc.tensor.matmul(scg[:], scpsi[:, pso:pso + 128],
                                     scpsi[:, 0:TA])
                    mhg = ps.tile([128, TA], F32, tag="mhg")
                    pho = TA + g * 256
                    nc.tensor.matmul(mhg[:, 0:256],
                                     mhpall[:, pho:pho + 128],
                                     mhpall[:, 0:256])
                    nc.tensor.matmul(mhg[:, 256:512],
                                     mhpall[:, pho + 128:pho + 256],
                                     mhpall[:, 256:512])
                    sc3g = scg[:].rearrange("p (t a) -> p t a", t=T, a=NA)
                    rming = gs.tile([128, T], F32, tag="rming")
                    nc.vector.tensor_reduce(rming[:], sc3g, axis=AX, op=OP.min)
                    # mhg columns are (half, anchor, tile-in-half) so the
                    # slot-min reduce reads contiguous tiles
                    # Hg = -(maha + PEN*(sc - rowmin)) (mhp is negated
                    # host-side); per-group per-anchor MAX of Hg = -Smin
                    Hg = gs.tile([128, TA], F32, tag="Hg")
                    for h in range(2):
                        rb = (rming[:, h * 8:(h + 1) * 8].unsqueeze(1)
                              .to_broadcast([128, NA, 8]))
                        sl = slice(h * 256, (h + 1) * 256)
                        nc.vector.scalar_tensor_tensor(
                            Hg[:, sl].rearrange("p (a u) -> p a u", a=NA, u=8),
                            rb, PEN,
                            mhg[:, sl].rearrange("p (a u) -> p a u", a=NA, u=8),
                            op0=OP.mult, op1=OP.add)
                    nc.vector.tensor_reduce(
                        smina[:, g * NA:(g + 1) * NA],
                        Hg[:].rearrange("p (h a u) -> p a h u",
                                        h=2, a=NA, u=8),
                        axis=mybir.AxisListType.XY, op=OP.max)
                    if g == 0:
                        # own verts: selected -maha = per-(h,u) max of Hg
                        # (non-argmin anchors are pushed down by PEN*delta)
                        S = an.tile([128, T], F32, tag="S")
                        nc.vector.tensor_reduce(
                            S[:],
                            Hg[:].rearrange("p (h a u) -> p h u a",
                                            h=2, a=NA, u=8),
                            axis=AX, op=OP.max)
                        nc.scalar.activation(W[:], S[:], EXP, scale=0.5)
                        # dsc = sc - rowmin, kept for the tail's norm select
                        rb2 = rming[:].unsqueeze(2).to_broadcast([128, T, NA])
                        nc.vector.tensor_tensor(
                            dsc[:].rearrange("p (t a) -> p t a", t=T, a=NA),
                            sc3g, rb2, op=OP.subtract)

                def emit_tile(t):
                    nch = len(chunks[t])
                    c16 = cnd.tile([128, 8 * nch], F32, tag="c16")
                    coff = offs[t]
                    for ci, cw in enumerate(chunks[t]):
                        pm = psm.tile([128, 512], F32, tag="pm")
                        nc.tensor.matmul(pm[:, :cw],
                                         candl[:, t * 128:(t + 1) * 128],
                                         candl[:, R + coff:R + coff + cw])
                        nc.vector.max(out=c16[:, ci * 8:(ci + 1) * 8],
                                      in_=pm[:, :cw])
                        coff += cw
                    if nch > 1:
                        top8 = cnd.tile([128, 8], F32, tag="top8")
                        nc.vector.max(out=top8[:], in_=c16[:])
                    else:
                        top8 = c16
                    # S5 = sum of top-K of -d^2 (negated at the end; the
                    # max(d^2, 0) clamp is dropped: hi/lo rounding is ~1e-6
                    # of the d^2 scale and the loss term is squared)
                    nc.vector.tensor_reduce(S5[:, t:t + 1], top8[:, :K],
                                            axis=AX, op=OP.add)

                emit_group(0)
                emit_tile(0)
                emit_tile(1)
                for g in range(1, 4):
                    emit_group(g)
                    emit_tile(g + 1)
                for g in range(4, n_cores):
                    emit_group(g)

                # -------- finalize group max (cross-partition roundtrip
                # hides behind the remaining main tiles)
                sming = tl.tile([128, NA], F32, tag="sming")
                nc.vector.tensor_reduce(
                    sming[:],
                    smina[:].rearrange("p (g a) -> p a g", g=n_cores, a=NA),
                    axis=AX, op=OP.max)
                sminr = tl.tile([1, NA], F32, tag="sminr")
                nc.gpsimd.tensor_reduce(sminr[:], sming[:],
                                        axis=mybir.AxisListType.C, op=OP.max)
                gmaxv = tl.tile([1, NA], F32, tag="gmaxv")
                nc.scalar.activation(gmaxv[:], sminr[:], EXP, scale=0.5)
                rnb = psx.tile([128, NA], F32, tag="rnb")

                for t in range(5, T - 2):
                    emit_tile(t)

                # -------- norm broadcast + per-vert weights (tail-pre)
                nrm = tl.tile([1, NA], F32, tag="nrm")
                nc.vector.tensor_scalar_max(nrm[:], gmaxv[:], 1.0)
                rn = tl.tile([1, NA], F32, tag="rn")
                nc.vector.reciprocal(rn[:], nrm[:])
                nc.tensor.matmul(rnb[:], ones[:], rn[:])
                # rnr = rn[argmin anchor] via the same penalty trick:
                # min over a of (PEN2 * (sc - rowmin) + rn[a])
                q = tl.tile([128, TA], F32, tag="q")
                q3 = q[:].rearrange("p (t a) -> p t a", t=T, a=NA)
                rnb_b = rnb[:].unsqueeze(1).to_broadcast([128, T, NA])
                nc.vector.scalar_tensor_tensor(
                    q3, dsc[:].rearrange("p (t a) -> p t a", t=T, a=NA),
                    PEN2, rnb_b, op0=OP.mult, op1=OP.add)
                rnr = tl.tile([128, T], F32, tag="rnr")
                nc.vector.tensor_reduce(rnr[:], q3, axis=AX, op=OP.min)
                wn = tl.tile([128, T], F32, tag="wn")
                nc.vector.tensor_mul(wn[:], W[:], rnr[:])
                # wf = wn * [wn > 0.01]; partials = sum -S5 * wf^2
                wf = tl.tile([128, T], F32, tag="wf")
                nc.vector.scalar_tensor_tensor(wf[:], wn[:], 0.01, wn[:],
                                               op0=OP.is_gt, op1=OP.mult)

                for t in range(T - 2, T):
                    emit_tile(t)

                zz = tl.tile([128, T], F32, tag="zz")
                nc.vector.tensor_mul(zz[:], wf[:], S5[:])
                term = tl.tile([128, T], F32, tag="term")
                prt = tl.tile([128, 1], F32, tag="prt")
                nc.vector.scalar_tensor_tensor(term[:], zz[:], -1.0, wf[:],
                                               op0=OP.mult, op1=OP.mult,
                                               accum_out=prt[:])
                nc.gpsimd.dma_start(part_d[:], prt[:, 0])
    return nc


_NC_CACHE = {}


def _prepare(inputs):
    verts = np.asarray(inputs["verts"], np.float32)
    anchor_verts = np.asarray(inputs["anchor_verts"], np.float32)
    obj_pts = np.asarray(inputs["obj_pts"], np.float32)
    cg = np.asarray(inputs["contact_gaussians"], np.float32)
    K = int(np.asarray(inputs["K"]))
    B, N, _ = verts.shape
    assert B == 1 and 1 <= K <= 8

    prep = _host_prep(verts, anchor_verts, obj_pts, cg, K)
    R = N // NCORES
    in_maps = [_pack_core(prep, c, R) for c in range(NCORES)]

    key = (R, prep["layout"], K)
    if key not in _NC_CACHE:
        _NC_CACHE[key] = _build_kernel(R=R, layout=prep["layout"], K=K,
                                       n_cores=NCORES)
    return _NC_CACHE[key], in_maps, prep


def kernel(**inputs) -> np.ndarray:
    nc, in_maps, prep = _prepare(inputs)
    res = run_bass_kernel_spmd(nc, in_maps, core_ids=list(range(NCORES)))
    total = np.float32(0.0)
    for c in range(NCORES):
        total += res.results[c]["part"].sum(dtype=np.float32)
    K = int(np.asarray(inputs["K"]))
    return np.float32(total / np.float32(prep["N"] * K))
